# revision 37
# baseline (speedup 1.0000x reference)
"""Trainium2 Bass kernel for nn_Conv_DCFD (dynamic conv filter decomposition).

Data-parallel over batch N=8 across 8 NeuronCores (one sample per core).

Per-sample pipeline (all shapes hardcoded):
  A. conv1 3x3 (C=128 -> 64) + folded BN + tanh  [PE tap-loop, bf16]
  B. conv2 3x3 (64 -> 72) + folded BN + tanh -> h2, in HALF row chunks
     (4 rows = one block pair) so the scatter pipeline starts early.
  C. basesT per 128-px block: h2_blk.T @ FBBD -> [128px, M, 2, 26] bf16
  D. yv per block: x_blk.T @ coef -> [128px, M, O] bf16 (PE filler work)
  E. per pair: GPSIMD local_scatter builds banded A^T rows (at2 [128,1536]);
     ONE XBAR DMA transpose converts cols [0:1152] into 9 ready lhsT chunks
     [128,12,128]-tile; PE transposes the remaining 3 chunks via PSUM.
     out_T[block] += a3[:,j,:].T @ yv[block+b-1] accumulated in PSUM.
  F. po -> obuf [128, NBLK, O]; chunked DMA out; host transposes + bias.
"""

import numpy as np
import ml_dtypes

N, C, H, W = 8, 128, 64, 64
O, KS, M, TEM, BS, INTER = 128, 5, 6, 12, 72, 64
EPS = 1e-5
PIX = H * W
NBLK = PIX // 128          # 32 blocks of 128 px (2 image rows each)
NPAIR = NBLK // 2          # 16 pairs (one conv2 half-chunk each)

# transpose split knob: how many of the 12 at2 chunks go via PE (rest via DMA)
PE_CHUNKS = 3

_f32 = np.float32
_bf16 = ml_dtypes.bfloat16

_cached = {}


def _host_prep(inputs):
    """Fold BN, rearrange weights; returns dict of per-core-shared arrays."""
    conv1_w = np.asarray(inputs["conv1_w"], _f32)
    conv1_b = np.asarray(inputs["conv1_b"], _f32)
    conv2_w = np.asarray(inputs["conv2_w"], _f32)
    conv2_b = np.asarray(inputs["conv2_b"], _f32)
    fb = np.asarray(inputs["fb_bases"], _f32)
    coef = np.asarray(inputs["coef"], _f32)

    s1 = np.asarray(inputs["bn1_gamma"], _f32) / np.sqrt(np.asarray(inputs["bn1_var"], _f32) + EPS)
    t1 = (conv1_b - np.asarray(inputs["bn1_mean"], _f32)) * s1 + np.asarray(inputs["bn1_beta"], _f32)
    s2 = np.asarray(inputs["bn2_gamma"], _f32) / np.sqrt(np.asarray(inputs["bn2_var"], _f32) + EPS)
    t2 = (conv2_b - np.asarray(inputs["bn2_mean"], _f32)) * s2 + np.asarray(inputs["bn2_beta"], _f32)

    w1T = np.ascontiguousarray(
        np.transpose(conv1_w.reshape(INTER, C, 9), (1, 2, 0))).astype(_bf16)  # [C,9,INTER]
    w2T = np.ascontiguousarray(
        np.transpose(conv2_w.reshape(BS, INTER, 9), (1, 2, 0))).astype(_bf16)  # [INTER,9,BS]

    FBBD = np.zeros((BS, M * 25), _f32)
    for m in range(M):
        FBBD[m * TEM:(m + 1) * TEM, m * 25:(m + 1) * 25] = fb

    coefT = np.zeros((C, M, O), _f32)
    for m in range(M):
        coefT[:, m, :] = coef[:, m::M].T

    idx = np.full((128, 26), -1, np.int16)
    for i in range(128):
        col = i % 64
        for dy in range(-2, 3):
            for dx in range(-2, 3):
                if 0 <= col + dx < 64:
                    idx[i, (dy + 2) * 5 + (dx + 2)] = i + 64 * dy + dx + 128
    idx2 = np.full((128, 52), -1, np.int16)
    idx2[:, 0:26] = idx
    idx2[:, 26:52] = np.where(idx >= 0, idx + 384, -1)
    idx4 = np.full((128, 104), -1, np.int16)
    idx4[:, 0:52] = idx2
    idx4[:, 52:104] = np.where(idx2 >= 0, idx2 + 768, -1)

    dummyidx = np.tile(np.array([0, 1], np.int16), (128, 1))

    return {
        "dummyidx": dummyidx,
        "w1T": w1T,
        "s1": s1.reshape(INTER, 1),
        "t1": t1.reshape(INTER, 1),
        "w2T": w2T,
        "s2": s2.reshape(BS, 1),
        "t2": t2.reshape(BS, 1),
        "fbbd": FBBD.astype(_bf16),
        "coefT": coefT.astype(_bf16),
        "idx4": idx4,
        "ident": np.eye(128, dtype=_bf16),
        "bias": np.asarray(inputs["bias"], _f32),
    }


def _build_program():
    import concourse.bass as bass
    import concourse.mybir as mybir
    import concourse.tile as tile
    from concourse import bacc

    f32 = mybir.dt.float32
    f32r = mybir.dt.float32r
    bf16 = mybir.dt.bfloat16
    i16 = mybir.dt.int16
    Tanh = mybir.ActivationFunctionType.Tanh

    nc = bacc.Bacc("TRN2", target_bir_lowering=False, debug=False, num_devices=8)

    xbf_d = nc.dram_tensor("xbf", [C, PIX], bf16, kind="ExternalInput").ap()
    w1_d = nc.dram_tensor("w1t", [C, 9 * INTER], bf16, kind="ExternalInput").ap()
    s1_d = nc.dram_tensor("s1", [INTER, 1], f32, kind="ExternalInput").ap()
    t1_d = nc.dram_tensor("t1", [INTER, 1], f32, kind="ExternalInput").ap()
    w2_d = nc.dram_tensor("w2t", [INTER, 9 * BS], bf16, kind="ExternalInput").ap()
    s2_d = nc.dram_tensor("s2", [BS, 1], f32, kind="ExternalInput").ap()
    t2_d = nc.dram_tensor("t2", [BS, 1], f32, kind="ExternalInput").ap()
    fbbd_d = nc.dram_tensor("fbbd", [BS, M * 25], bf16, kind="ExternalInput").ap()
    coef_d = nc.dram_tensor("coeft", [C, M * O], bf16, kind="ExternalInput").ap()
    idx_d = nc.dram_tensor("idx4", [128, 104], i16, kind="ExternalInput").ap()
    didx_d = nc.dram_tensor("dummyidx", [128, 2], i16, kind="ExternalInput").ap()
    ident_d = nc.dram_tensor("ident", [128, 128], bf16, kind="ExternalInput").ap()
    out_d = nc.dram_tensor("out", [PIX, O], f32, kind="ExternalOutput").ap()

    taps = [(a, b) for a in range(3) for b in range(3)]

    from contextlib import ExitStack

    with tile.TileContext(nc) as tc, ExitStack() as stack:
        consts = stack.enter_context(tc.tile_pool(name="consts", bufs=1))
        apool = stack.enter_context(tc.tile_pool(name="apool", bufs=15))
        a3dpool = stack.enter_context(tc.tile_pool(name="a3dpool", bufs=14))
        a3ppool = stack.enter_context(tc.tile_pool(name="a3ppool", bufs=16))
        yvpool = stack.enter_context(tc.tile_pool(name="yvpool", bufs=16))

        # ---- inputs into SBUF.  scalar hwdge ring carries params (dummyidx +
        # idxt first: a tiny dummy scatter right after triggers the GPSIMD
        # library load early, off the critical path); sync ring carries xbf
        # and later the XBAR transposes.  conv1 reads xbf directly (no padded
        # copy): border taps use clipped row/col windows. ----
        didx = consts.tile([128, 2], i16)
        nc.scalar.dma_start(out=didx, in_=didx_d)
        idxt = consts.tile([128, 104], i16)
        nc.scalar.dma_start(out=idxt, in_=idx_d)
        obuf = consts.tile([128, NBLK, O], f32)
        # dummy scatter: forces LOAD_LIB now (waits only the didx DMA)
        nc.gpsimd.local_scatter(
            obuf[:, 0, 0:1].bitcast(i16),
            didx[:],
            didx[:],
            channels=128,
            num_elems=2,
            num_idxs=2,
        )
        w2 = consts.tile([INTER, 9, BS], bf16)
        nc.scalar.dma_start(out=w2, in_=w2_d.rearrange("c (t o) -> c t o", t=9))
        s1 = consts.tile([INTER, 1], f32)
        nc.scalar.dma_start(out=s1, in_=s1_d)
        t1 = consts.tile([INTER, 1], f32)
        nc.scalar.dma_start(out=t1, in_=t1_d)
        xbf = consts.tile([C, NBLK, 128], bf16)
        xbf_src = xbf_d.rearrange("c (b p) -> c b p", b=NBLK)
        ident = consts.tile([128, 128], bf16)
        nc.sync.dma_start(out=xbf[:, 0:3, :], in_=xbf_src[:, 0:3, :])
        nc.sync.dma_start(out=ident, in_=ident_d)
        w1 = consts.tile([C, 9, INTER], bf16)
        nc.sync.dma_start(out=w1, in_=w1_d.rearrange("c (t o) -> c t o", t=9))
        nc.sync.dma_start(out=xbf[:, 3:8, :], in_=xbf_src[:, 3:8, :])
        coefT = consts.tile([C, M, O], bf16)
        nc.sync.dma_start(out=coefT, in_=coef_d.rearrange("c (m o) -> c m o", m=M))
        nc.sync.dma_start(out=xbf[:, 8:16, :], in_=xbf_src[:, 8:16, :])
        s2 = consts.tile([BS, 1], f32)
        nc.sync.dma_start(out=s2, in_=s2_d)
        t2 = consts.tile([BS, 1], f32)
        nc.sync.dma_start(out=t2, in_=t2_d)
        fbbd = consts.tile([BS, M * 25], bf16)
        nc.sync.dma_start(out=fbbd, in_=fbbd_d)
        nc.sync.dma_start(out=xbf[:, 16:24, :], in_=xbf_src[:, 16:24, :])
        nc.sync.dma_start(out=xbf[:, 24:32, :], in_=xbf_src[:, 24:32, :])

        h1p = consts.tile([INTER, 66, 66], bf16)
        h2 = consts.tile([BS, PIX], bf16)
        basesT = consts.tile([128, NPAIR, M, 2, 26], bf16)
        zero_y = consts.tile([128, M, O], bf16)
        nc.vector.memset(zero_y, 0.0)
        # zero h1p borders (interior overwritten by conv1)
        nc.vector.memset(h1p[:, 0, :].bitcast(bf16), 0.0)
        nc.vector.memset(h1p[:, 65, :].bitcast(bf16), 0.0)
        nc.vector.memset(h1p[:, :, 0].bitcast(bf16), 0.0)
        nc.vector.memset(h1p[:, :, 65].bitcast(bf16), 0.0)

        psA = stack.enter_context(tc.tile_pool(name="psA", bufs=2, space="PSUM"))
        psT = stack.enter_context(tc.tile_pool(name="psT", bufs=2, space="PSUM"))
        psY = stack.enter_context(tc.tile_pool(name="psY", bufs=1, space="PSUM"))
        psB = stack.enter_context(tc.tile_pool(name="psB", bufs=1, space="PSUM"))
        psO = stack.enter_context(tc.tile_pool(name="psO", bufs=2, space="PSUM"))

        # ---- state ----
        yv_blocks = [None] * NBLK
        at2s = [[None] * 3 for _ in range(NPAIR)]
        a3gs = [[None] * 3 for _ in range(NPAIR)]
        ring_toggle = [0]

        def ytv(i):
            # y tile for band source block index i-1 (i in 0..NBLK+1)
            if i == 0 or i == NBLK + 1:
                return zero_y[:]
            return yv_blocks[i - 1][:]

        def d_chunk(b0, b1, pool=None):
            for B in range(b0, b1):
                yvt = yvpool.tile([128, M, O], bf16, tag="yv")
                for h in range(2):
                    py = ((psO.tile([128, 3 * O], f32, tag="po", name="py")
                           if pool is not None else
                           psY.tile([128, 3 * O], f32, tag="py", name="py")))
                    nc.tensor.matmul(
                        py[:],
                        lhsT=xbf[:, B, :],
                        rhs=coefT[:, 3 * h: 3 * h + 3, :].rearrange("c m o -> c (m o)"),
                        start=True,
                        stop=True,
                    )
                    dst = yvt[:, 3 * h: 3 * h + 3, :].rearrange("p m o -> p (m o)")
                    nc.vector.tensor_copy(dst, py[:])
                yv_blocks[B] = yvt

        xbfv = xbf[:].rearrange("c b (s w) -> c (b s) w", s=2)  # [C, 64, 64]
        # tap order: full center tap first (start=True zeroes the whole tile)
        taps1 = [(1, 1)] + [t for t in taps if t != (1, 1)]

        def conv1_rows(r0, nrows):
            p1 = psA.tile([INTER, 8, 64], f32, tag="conv", name="p1")
            for t, (a, b) in enumerate(taps1):
                di, dj = a - 1, b - 1
                lo, hi = max(0, r0 + di), min(63, r0 + nrows - 1 + di)
                jlo, jhi = max(0, dj), min(63, 63 + dj)
                nc.tensor.matmul(
                    p1[:, lo - di - r0: hi - di - r0 + 1, jlo - dj: jhi - dj + 1],
                    lhsT=w1[:, 3 * a + b, :],
                    rhs=xbfv[:, lo: hi + 1, jlo: jhi + 1],
                    start=(t == 0),
                    stop=(t == 8),
                )
            nc.scalar.activation(
                h1p[:, 1 + r0: 1 + r0 + nrows, 1:65],
                p1[:, 0:nrows, :],
                Tanh,
                bias=t1[:],
                scale=s1[:],
            )

        def conv1(r):
            conv1_rows(8 * r, 8)

        def conv2h(rh):
            # half chunk: 4 image rows = blocks 2rh, 2rh+1 (pair rh)
            p2 = psA.tile([BS, 256], f32, tag="conv", name="p2")
            for t, (a, b) in enumerate(taps):
                nc.tensor.matmul(
                    p2[:],
                    lhsT=w2[:, t, :],
                    rhs=h1p[:, a + 4 * rh: a + 4 * rh + 4, b: b + 64],
                    start=(t == 0),
                    stop=(t == 8),
                )
            nc.scalar.activation(
                h2[:, 256 * rh: 256 * (rh + 1)],
                p2[:],
                Tanh,
                bias=t2[:],
                scale=s2[:],
            )

        def do_C2(P):
            # both blocks of pair P into one psum tile (1.2KB, fits a bank):
            # no WAR between halves, one combined copy
            pb = psB.tile([128, 2, M * 25], f32, tag="pb", name="pb2")
            for half in range(2):
                nc.tensor.matmul(
                    pb[:, half, :],
                    lhsT=h2[:, 128 * (2 * P + half): 128 * (2 * P + half + 1)],
                    rhs=fbbd[:],
                    start=True,
                    stop=True,
                )
            dst = basesT[:, P, :, :, 0:25]
            src = pb[:].rearrange("p h (m l) -> p m h l", m=M)
            if P < 2 or P % 2 == 1:
                nc.vector.tensor_copy(dst, src)
            else:
                nc.scalar.copy(dst, src)

        def scatter_pair(P):
            # GPSIMD: banded A^T rows for pair P (blocks 2P, 2P+1), 2 m per call
            tiles = []
            for mb in range(3):
                at2 = apool.tile([128, 1536], bf16, tag="at")
                nc.gpsimd.local_scatter(
                    at2[:],
                    basesT[:, P, 2 * mb: 2 * mb + 2, :, :].rearrange(
                        "p m b l -> p (m b l)"),
                    idxt[:],
                    channels=128,
                    num_elems=1536,
                    num_idxs=104,
                )
                tiles.append(at2)
            at2s[P] = tiles

        def expand_pair(P, pe_chunks=PE_CHUNKS):
            # transpose pair P's at2 tiles into 12 lhsT chunks [128,128] each
            # (chunk j = dm*6 + half*3 + b).  First 12-pe_chunks chunks via one
            # XBAR DMA transpose into a3d; the rest via PE transpose + psum
            # bounce + DVE copy into a3p tiles (single producer per tile).
            dma_chunks = 12 - pe_chunks
            for mb in range(3):
                at2 = at2s[P][mb]
                chunk_ap = [None] * 12
                if dma_chunks > 0:
                    a3d = a3dpool.tile([128, dma_chunks, 128], bf16,
                                       tag="a3d", name="a3d")
                    ring_toggle[0] += 1
                    nc.sync.dma_start_transpose(
                        a3d[:], at2[:, 0:128 * dma_chunks])
                    for j in range(dma_chunks):
                        chunk_ap[j] = a3d[:, j, :]
                for c0 in range(dma_chunks, 12, 3):
                    pt = psT.tile([128, 3, 128], bf16, tag="pt")
                    for c in range(3):
                        nc.tensor.transpose(
                            pt[:, c, :],
                            at2[:, 128 * (c0 + c): 128 * (c0 + c + 1)],
                            ident[:],
                        )
                    a3p = a3ppool.tile([128, 3, 128], bf16, tag="a3p",
                                       name="a3p")
                    nc.vector.tensor_copy(a3p[:], pt[:])
                    for c in range(3):
                        chunk_ap[c0 + c] = a3p[:, c, :]
                a3gs[P][mb] = chunk_ap

        po4 = {}
        out_flush = {7: (0, 8), 15: (8, 16), 23: (16, 24), 27: (24, 28),
                     29: (28, 30), 31: (30, 32)}

        def emit_banded(B):
            j4 = B % 4
            if j4 == 0:
                po4["t"] = psO.tile([128, 4, O], f32, tag="po", name="po4")
            po = po4["t"]
            P = B // 2
            half = B % 2
            for m in range(M):
                mb, dm = m // 2, m % 2
                chunks = a3gs[P][mb]
                for b in range(3):
                    nc.tensor.matmul(
                        po[:, j4, :],
                        lhsT=chunks[dm * 6 + half * 3 + b],
                        rhs=ytv(B + b)[:, m, :],
                        start=(m == 0 and b == 0),
                        stop=(m == M - 1 and b == 2),
                    )
            if B == 29:
                # last 4-block group: copy + flush in 2-block halves so the
                # final DMA starts earlier
                nc.vector.tensor_copy(obuf[:, 28:30, :], po[:, 0:2, :])
            elif B == 31:
                nc.vector.tensor_copy(obuf[:, 30:32, :], po[:, 2:4, :])
            elif j4 == 3:
                nc.scalar.copy(obuf[:, B - 3: B + 1, :], po[:])
            if B in out_flush:
                k0, k1 = out_flush[B]
                eng = nc.scalar if B == 31 else nc.sync
                eng.dma_start(
                    out=out_d[128 * k0: 128 * k1, :].rearrange(
                        "(b p) o -> p b o", p=128),
                    in_=obuf[:, k0:k1, :],
                )

        # ---- schedule ----
        # PE warmup: throwaway matmuls on ident while xbf/w1 DMAs land, so the
        # latency-critical conv1->scatter(0) chain starts at a ramped p-state.
        wps = psB.tile([128, 128], f32, tag="pb", name="wps")
        for _ in range(60):
            nc.tensor.matmul(wps[:], lhsT=ident[:], rhs=ident[:],
                             start=True, stop=True)
        nc.vector.tensor_copy(obuf[:, 0, :], wps[:])

        # prologue: fast path to scatter(0); bases production (conv2h + do_C)
        # then runs TWO PAIRS AHEAD of the scatters so GPSIMD never starves.
        conv1_rows(0, 5)
        conv2h(0)
        do_C2(0)
        scatter_pair(0)
        conv1_rows(5, 3)
        conv1(1)
        conv2h(1)
        do_C2(1)
        scatter_pair(1)
        conv2h(2)
        do_C2(2)
        conv1(2)
        conv2h(3)
        do_C2(3)
        d_chunk(0, 2, pool=psO)
        d_chunk(2, 4)
        expand_pair(0)

        conv1_done = [2]

        def need_conv1(p):
            k = min(4 * p + 4, 63) // 8
            while conv1_done[0] < k:
                conv1_done[0] += 1
                conv1(conv1_done[0])

        for rh in range(2, NPAIR):
            scatter_pair(rh)
            # issue DMA transposes for the previous pair early (latency
            # hiding); late pairs lean more on PE (ring backlog at the end)
            expand_pair(rh - 1, pe_chunks=6 if rh >= 13 else PE_CHUNKS)
            # produce pair rh+2 (two ahead of the scatter queue: absorbs the
            # conv1-bunched iterations without starving GPSIMD)
            if rh + 2 < NPAIR:
                need_conv1(min(rh + 3, NPAIR - 1))
                conv2h(rh + 2)
                do_C2(rh + 2)
            # emits lag expansion by one pair
            emit_banded(2 * rh - 4)
            emit_banded(2 * rh - 3)
            d_chunk(2 * rh, 2 * rh + 2)
        # tail: pair-14 emits first (they only need pair-14 a3, already
        # expanded), overlapping the wait for scatter(15)
        emit_banded(NBLK - 4)
        emit_banded(NBLK - 3)
        expand_pair(NPAIR - 1, pe_chunks=12)  # last pair all-PE: shortest latency
        emit_banded(NBLK - 2)
        emit_banded(NBLK - 1)

    nc.compile()
    return nc


def _get_program():
    if "nc" not in _cached:
        _cached["nc"] = _build_program()
    return _cached["nc"]


def _build_in_maps(inputs, prep=None):
    if prep is None:
        prep = _host_prep(inputs)
    x = np.asarray(inputs["x"], _f32)

    shared = {
        "w1t": np.ascontiguousarray(prep["w1T"].reshape(C, 9 * INTER)),
        "s1": prep["s1"], "t1": prep["t1"],
        "w2t": np.ascontiguousarray(prep["w2T"].reshape(INTER, 9 * BS)),
        "s2": prep["s2"], "t2": prep["t2"],
        "fbbd": prep["fbbd"],
        "coeft": np.ascontiguousarray(prep["coefT"].reshape(C, M * O)),
        "idx4": prep["idx4"],
        "dummyidx": prep["dummyidx"],
        "ident": prep["ident"],
    }

    in_maps = []
    for n in range(N):
        m = dict(shared)
        m["xbf"] = np.ascontiguousarray(x[n].reshape(C, PIX).astype(_bf16))
        in_maps.append(m)
    return in_maps


def kernel(**inputs):
    from concourse.bass_utils import run_bass_kernel_spmd

    prep = _host_prep(inputs)
    in_maps = _build_in_maps(inputs, prep)

    nc = _get_program()
    res = run_bass_kernel_spmd(nc, in_maps, core_ids=list(range(N)))

    out = np.zeros((N, O, H, W), _f32)
    bias = prep["bias"]
    for n in range(N):
        outT = res.results[n]["out"]            # [4096, 128]
        out[n] = (outT.T + bias[:, None]).reshape(O, H, W)
    return out


# revision 38
# speedup vs baseline: 1.0264x; 1.0264x over previous
"""Trainium2 Bass kernel for nn_Conv_DCFD (dynamic conv filter decomposition).

Data-parallel over batch N=8 across 8 NeuronCores (one sample per core).

Per-sample pipeline (all shapes hardcoded):
  A. conv1 3x3 (C=128 -> 64) + folded BN + tanh  [PE tap-loop, bf16]
  B. conv2 3x3 (64 -> 72) + folded BN + tanh -> h2, in HALF row chunks
     (4 rows = one block pair) so the scatter pipeline starts early.
  C. basesT per 128-px block: h2_blk.T @ FBBD -> [128px, M, 2, 26] bf16
  D. yv per block: x_blk.T @ coef -> [128px, M, O] bf16 (PE filler work)
  E. per pair: GPSIMD local_scatter builds banded A^T rows (at2 [128,1536]);
     ONE XBAR DMA transpose converts cols [0:1152] into 9 ready lhsT chunks
     [128,12,128]-tile; PE transposes the remaining 3 chunks via PSUM.
     out_T[block] += a3[:,j,:].T @ yv[block+b-1] accumulated in PSUM.
  F. po -> obuf [128, NBLK, O]; chunked DMA out; host transposes + bias.
"""

import numpy as np
import ml_dtypes

N, C, H, W = 8, 128, 64, 64
O, KS, M, TEM, BS, INTER = 128, 5, 6, 12, 72, 64
EPS = 1e-5
PIX = H * W
NBLK = PIX // 128          # 32 blocks of 128 px (2 image rows each)
NPAIR = NBLK // 2          # 16 pairs (one conv2 half-chunk each)

# transpose split knob: how many of the 12 at2 chunks go via PE (rest via DMA)
PE_CHUNKS = 3

_f32 = np.float32
_bf16 = ml_dtypes.bfloat16

_cached = {}


def _host_prep(inputs):
    """Fold BN, rearrange weights; returns dict of per-core-shared arrays."""
    conv1_w = np.asarray(inputs["conv1_w"], _f32)
    conv1_b = np.asarray(inputs["conv1_b"], _f32)
    conv2_w = np.asarray(inputs["conv2_w"], _f32)
    conv2_b = np.asarray(inputs["conv2_b"], _f32)
    fb = np.asarray(inputs["fb_bases"], _f32)
    coef = np.asarray(inputs["coef"], _f32)

    s1 = np.asarray(inputs["bn1_gamma"], _f32) / np.sqrt(np.asarray(inputs["bn1_var"], _f32) + EPS)
    t1 = (conv1_b - np.asarray(inputs["bn1_mean"], _f32)) * s1 + np.asarray(inputs["bn1_beta"], _f32)
    s2 = np.asarray(inputs["bn2_gamma"], _f32) / np.sqrt(np.asarray(inputs["bn2_var"], _f32) + EPS)
    t2 = (conv2_b - np.asarray(inputs["bn2_mean"], _f32)) * s2 + np.asarray(inputs["bn2_beta"], _f32)

    w1T = np.ascontiguousarray(
        np.transpose(conv1_w.reshape(INTER, C, 9), (1, 2, 0))).astype(_bf16)  # [C,9,INTER]
    w2T = np.ascontiguousarray(
        np.transpose(conv2_w.reshape(BS, INTER, 9), (1, 2, 0))).astype(_bf16)  # [INTER,9,BS]

    FBBD = np.zeros((BS, M * 25), _f32)
    for m in range(M):
        FBBD[m * TEM:(m + 1) * TEM, m * 25:(m + 1) * 25] = fb

    coefT = np.zeros((C, M, O), _f32)
    for m in range(M):
        coefT[:, m, :] = coef[:, m::M].T

    idx = np.full((128, 26), -1, np.int16)
    for i in range(128):
        col = i % 64
        for dy in range(-2, 3):
            for dx in range(-2, 3):
                if 0 <= col + dx < 64:
                    idx[i, (dy + 2) * 5 + (dx + 2)] = i + 64 * dy + dx + 128
    idx2 = np.full((128, 52), -1, np.int16)
    idx2[:, 0:26] = idx
    idx2[:, 26:52] = np.where(idx >= 0, idx + 384, -1)
    idx4 = np.full((128, 104), -1, np.int16)
    idx4[:, 0:52] = idx2
    idx4[:, 52:104] = np.where(idx2 >= 0, idx2 + 768, -1)

    dummyidx = np.tile(np.array([0, 1], np.int16), (128, 1))

    return {
        "dummyidx": dummyidx,
        "w1T": w1T,
        "s1": s1.reshape(INTER, 1),
        "t1": t1.reshape(INTER, 1),
        "w2T": w2T,
        "s2": s2.reshape(BS, 1),
        "t2": t2.reshape(BS, 1),
        "fbbd": FBBD.astype(_bf16),
        "coefT": coefT.astype(_bf16),
        "idx4": idx4,
        "ident": np.eye(128, dtype=_bf16),
        "bias": np.asarray(inputs["bias"], _f32),
    }


def _build_program():
    import concourse.bass as bass
    import concourse.mybir as mybir
    import concourse.tile as tile
    from concourse import bacc

    f32 = mybir.dt.float32
    f32r = mybir.dt.float32r
    bf16 = mybir.dt.bfloat16
    i16 = mybir.dt.int16
    Tanh = mybir.ActivationFunctionType.Tanh

    nc = bacc.Bacc("TRN2", target_bir_lowering=False, debug=False, num_devices=8)

    xbf_d = nc.dram_tensor("xbf", [C, PIX], bf16, kind="ExternalInput").ap()
    w1_d = nc.dram_tensor("w1t", [C, 9 * INTER], bf16, kind="ExternalInput").ap()
    s1_d = nc.dram_tensor("s1", [INTER, 1], f32, kind="ExternalInput").ap()
    t1_d = nc.dram_tensor("t1", [INTER, 1], f32, kind="ExternalInput").ap()
    w2_d = nc.dram_tensor("w2t", [INTER, 9 * BS], bf16, kind="ExternalInput").ap()
    s2_d = nc.dram_tensor("s2", [BS, 1], f32, kind="ExternalInput").ap()
    t2_d = nc.dram_tensor("t2", [BS, 1], f32, kind="ExternalInput").ap()
    fbbd_d = nc.dram_tensor("fbbd", [BS, M * 25], bf16, kind="ExternalInput").ap()
    coef_d = nc.dram_tensor("coeft", [C, M * O], bf16, kind="ExternalInput").ap()
    idx_d = nc.dram_tensor("idx4", [128, 104], i16, kind="ExternalInput").ap()
    didx_d = nc.dram_tensor("dummyidx", [128, 2], i16, kind="ExternalInput").ap()
    ident_d = nc.dram_tensor("ident", [128, 128], bf16, kind="ExternalInput").ap()
    out_d = nc.dram_tensor("out", [PIX, O], f32, kind="ExternalOutput").ap()

    taps = [(a, b) for a in range(3) for b in range(3)]

    from contextlib import ExitStack

    with tile.TileContext(nc) as tc, ExitStack() as stack:
        consts = stack.enter_context(tc.tile_pool(name="consts", bufs=1))
        apool = stack.enter_context(tc.tile_pool(name="apool", bufs=15))
        a3dpool = stack.enter_context(tc.tile_pool(name="a3dpool", bufs=14))
        a3ppool = stack.enter_context(tc.tile_pool(name="a3ppool", bufs=16))
        yvpool = stack.enter_context(tc.tile_pool(name="yvpool", bufs=16))

        # ---- inputs into SBUF.  scalar hwdge ring carries params (dummyidx +
        # idxt first: a tiny dummy scatter right after triggers the GPSIMD
        # library load early, off the critical path); sync ring carries xbf
        # and later the XBAR transposes.  conv1 reads xbf directly (no padded
        # copy): border taps use clipped row/col windows. ----
        didx = consts.tile([128, 2], i16)
        nc.scalar.dma_start(out=didx, in_=didx_d)
        idxt = consts.tile([128, 104], i16)
        nc.scalar.dma_start(out=idxt, in_=idx_d)
        obuf = consts.tile([128, NBLK, O], f32)
        # dummy scatter: forces LOAD_LIB now (waits only the didx DMA)
        nc.gpsimd.local_scatter(
            obuf[:, 0, 0:1].bitcast(i16),
            didx[:],
            didx[:],
            channels=128,
            num_elems=2,
            num_idxs=2,
        )
        w2 = consts.tile([INTER, 9, BS], bf16)
        nc.scalar.dma_start(out=w2, in_=w2_d.rearrange("c (t o) -> c t o", t=9))
        s1 = consts.tile([INTER, 1], f32)
        nc.scalar.dma_start(out=s1, in_=s1_d)
        t1 = consts.tile([INTER, 1], f32)
        nc.scalar.dma_start(out=t1, in_=t1_d)
        xbf = consts.tile([C, NBLK, 128], bf16)
        xbf_src = xbf_d.rearrange("c (b p) -> c b p", b=NBLK)
        ident = consts.tile([128, 128], bf16)
        nc.sync.dma_start(out=ident, in_=ident_d)
        nc.sync.dma_start(out=xbf[:, 0:3, :], in_=xbf_src[:, 0:3, :])
        w1 = consts.tile([C, 9, INTER], bf16)
        nc.sync.dma_start(out=w1, in_=w1_d.rearrange("c (t o) -> c t o", t=9))
        nc.sync.dma_start(out=xbf[:, 3:8, :], in_=xbf_src[:, 3:8, :])
        coefT = consts.tile([C, M, O], bf16)
        nc.sync.dma_start(out=coefT, in_=coef_d.rearrange("c (m o) -> c m o", m=M))
        nc.sync.dma_start(out=xbf[:, 8:16, :], in_=xbf_src[:, 8:16, :])
        s2 = consts.tile([BS, 1], f32)
        nc.sync.dma_start(out=s2, in_=s2_d)
        t2 = consts.tile([BS, 1], f32)
        nc.sync.dma_start(out=t2, in_=t2_d)
        fbbd = consts.tile([BS, M * 25], bf16)
        nc.sync.dma_start(out=fbbd, in_=fbbd_d)
        nc.sync.dma_start(out=xbf[:, 16:24, :], in_=xbf_src[:, 16:24, :])
        nc.sync.dma_start(out=xbf[:, 24:32, :], in_=xbf_src[:, 24:32, :])

        h1p = consts.tile([INTER, 66, 66], bf16)
        h2 = consts.tile([BS, PIX], bf16)
        basesT = consts.tile([128, NPAIR, M, 2, 26], bf16)
        zero_y = consts.tile([128, M, O], bf16)
        nc.vector.memset(zero_y, 0.0)
        # zero h1p borders (interior overwritten by conv1)
        nc.vector.memset(h1p[:, 0, :].bitcast(bf16), 0.0)
        nc.vector.memset(h1p[:, 65, :].bitcast(bf16), 0.0)
        nc.vector.memset(h1p[:, :, 0].bitcast(bf16), 0.0)
        nc.vector.memset(h1p[:, :, 65].bitcast(bf16), 0.0)

        psA = stack.enter_context(tc.tile_pool(name="psA", bufs=2, space="PSUM"))
        psT = stack.enter_context(tc.tile_pool(name="psT", bufs=2, space="PSUM"))
        psY = stack.enter_context(tc.tile_pool(name="psY", bufs=1, space="PSUM"))
        psB = stack.enter_context(tc.tile_pool(name="psB", bufs=1, space="PSUM"))
        psO = stack.enter_context(tc.tile_pool(name="psO", bufs=2, space="PSUM"))

        # ---- state ----
        yv_blocks = [None] * NBLK
        at2s = [[None] * 3 for _ in range(NPAIR)]
        a3gs = [[None] * 3 for _ in range(NPAIR)]
        ring_toggle = [0]

        def ytv(i):
            # y tile for band source block index i-1 (i in 0..NBLK+1)
            if i == 0 or i == NBLK + 1:
                return zero_y[:]
            return yv_blocks[i - 1][:]

        def d_chunk(b0, b1, pool=None):
            for B in range(b0, b1):
                yvt = yvpool.tile([128, M, O], bf16, tag="yv")
                for h in range(2):
                    py = ((psO.tile([128, 3 * O], f32, tag="po", name="py")
                           if pool is not None else
                           psY.tile([128, 3 * O], f32, tag="py", name="py")))
                    nc.tensor.matmul(
                        py[:],
                        lhsT=xbf[:, B, :],
                        rhs=coefT[:, 3 * h: 3 * h + 3, :].rearrange("c m o -> c (m o)"),
                        start=True,
                        stop=True,
                    )
                    dst = yvt[:, 3 * h: 3 * h + 3, :].rearrange("p m o -> p (m o)")
                    nc.vector.tensor_copy(dst, py[:])
                yv_blocks[B] = yvt

        xbfv = xbf[:].rearrange("c b (s w) -> c (b s) w", s=2)  # [C, 64, 64]
        # tap order: full center tap first (start=True zeroes the whole tile)
        taps1 = [(1, 1)] + [t for t in taps if t != (1, 1)]

        def conv1_rows(r0, nrows):
            p1 = psA.tile([INTER, 8, 64], f32, tag="conv", name="p1")
            for t, (a, b) in enumerate(taps1):
                di, dj = a - 1, b - 1
                lo, hi = max(0, r0 + di), min(63, r0 + nrows - 1 + di)
                jlo, jhi = max(0, dj), min(63, 63 + dj)
                nc.tensor.matmul(
                    p1[:, lo - di - r0: hi - di - r0 + 1, jlo - dj: jhi - dj + 1],
                    lhsT=w1[:, 3 * a + b, :],
                    rhs=xbfv[:, lo: hi + 1, jlo: jhi + 1],
                    start=(t == 0),
                    stop=(t == 8),
                )
            nc.scalar.activation(
                h1p[:, 1 + r0: 1 + r0 + nrows, 1:65],
                p1[:, 0:nrows, :],
                Tanh,
                bias=t1[:],
                scale=s1[:],
            )

        def conv1(r):
            conv1_rows(8 * r, 8)

        def conv2h(rh):
            # half chunk: 4 image rows = blocks 2rh, 2rh+1 (pair rh)
            p2 = psA.tile([BS, 256], f32, tag="conv", name="p2")
            for t, (a, b) in enumerate(taps):
                nc.tensor.matmul(
                    p2[:],
                    lhsT=w2[:, t, :],
                    rhs=h1p[:, a + 4 * rh: a + 4 * rh + 4, b: b + 64],
                    start=(t == 0),
                    stop=(t == 8),
                )
            nc.scalar.activation(
                h2[:, 256 * rh: 256 * (rh + 1)],
                p2[:],
                Tanh,
                bias=t2[:],
                scale=s2[:],
            )

        def do_C2(P):
            # both blocks of pair P into one psum tile (1.2KB, fits a bank):
            # no WAR between halves, one combined copy
            pb = psB.tile([128, 2, M * 25], f32, tag="pb", name="pb2")
            for half in range(2):
                nc.tensor.matmul(
                    pb[:, half, :],
                    lhsT=h2[:, 128 * (2 * P + half): 128 * (2 * P + half + 1)],
                    rhs=fbbd[:],
                    start=True,
                    stop=True,
                )
            dst = basesT[:, P, :, :, 0:25]
            src = pb[:].rearrange("p h (m l) -> p m h l", m=M)
            if P < 2 or P % 2 == 1:
                nc.vector.tensor_copy(dst, src)
            else:
                nc.scalar.copy(dst, src)

        def scatter_pair(P):
            # GPSIMD: banded A^T rows for pair P (blocks 2P, 2P+1), 2 m per call
            tiles = []
            for mb in range(3):
                at2 = apool.tile([128, 1536], bf16, tag="at")
                nc.gpsimd.local_scatter(
                    at2[:],
                    basesT[:, P, 2 * mb: 2 * mb + 2, :, :].rearrange(
                        "p m b l -> p (m b l)"),
                    idxt[:],
                    channels=128,
                    num_elems=1536,
                    num_idxs=104,
                )
                tiles.append(at2)
            at2s[P] = tiles

        def expand_pair(P, pe_chunks=PE_CHUNKS):
            # transpose pair P's at2 tiles into 12 lhsT chunks [128,128] each
            # (chunk j = dm*6 + half*3 + b).  First 12-pe_chunks chunks via one
            # XBAR DMA transpose into a3d; the rest via PE transpose + psum
            # bounce + DVE copy into a3p tiles (single producer per tile).
            dma_chunks = 12 - pe_chunks
            for mb in range(3):
                at2 = at2s[P][mb]
                chunk_ap = [None] * 12
                if dma_chunks > 0:
                    a3d = a3dpool.tile([128, dma_chunks, 128], bf16,
                                       tag="a3d", name="a3d")
                    ring_toggle[0] += 1
                    nc.sync.dma_start_transpose(
                        a3d[:], at2[:, 0:128 * dma_chunks])
                    for j in range(dma_chunks):
                        chunk_ap[j] = a3d[:, j, :]
                for c0 in range(dma_chunks, 12, 3):
                    pt = psT.tile([128, 3, 128], bf16, tag="pt")
                    for c in range(3):
                        nc.tensor.transpose(
                            pt[:, c, :],
                            at2[:, 128 * (c0 + c): 128 * (c0 + c + 1)],
                            ident[:],
                        )
                    a3p = a3ppool.tile([128, 3, 128], bf16, tag="a3p",
                                       name="a3p")
                    nc.vector.tensor_copy(a3p[:], pt[:])
                    for c in range(3):
                        chunk_ap[c0 + c] = a3p[:, c, :]
                a3gs[P][mb] = chunk_ap

        po4 = {}
        out_flush = {7: (0, 8), 15: (8, 16), 23: (16, 24), 27: (24, 28),
                     29: (28, 30), 31: (30, 32)}

        def emit_banded(B):
            j4 = B % 4
            if j4 == 0:
                po4["t"] = psO.tile([128, 4, O], f32, tag="po", name="po4")
            po = po4["t"]
            P = B // 2
            half = B % 2
            for m in range(M):
                mb, dm = m // 2, m % 2
                chunks = a3gs[P][mb]
                for b in range(3):
                    nc.tensor.matmul(
                        po[:, j4, :],
                        lhsT=chunks[dm * 6 + half * 3 + b],
                        rhs=ytv(B + b)[:, m, :],
                        start=(m == 0 and b == 0),
                        stop=(m == M - 1 and b == 2),
                    )
            if B == 29:
                # last 4-block group: copy + flush in 2-block halves so the
                # final DMA starts earlier
                nc.vector.tensor_copy(obuf[:, 28:30, :], po[:, 0:2, :])
            elif B == 31:
                nc.vector.tensor_copy(obuf[:, 30:32, :], po[:, 2:4, :])
            elif j4 == 3:
                nc.scalar.copy(obuf[:, B - 3: B + 1, :], po[:])
            if B in out_flush:
                k0, k1 = out_flush[B]
                eng = nc.scalar if B == 31 else nc.sync
                eng.dma_start(
                    out=out_d[128 * k0: 128 * k1, :].rearrange(
                        "(b p) o -> p b o", p=128),
                    in_=obuf[:, k0:k1, :],
                )

        # ---- schedule ----
        # PE warmup: throwaway matmuls on ident while xbf/w1 DMAs land, so the
        # latency-critical conv1->scatter(0) chain starts at a ramped p-state.
        wps = psB.tile([128, 128], f32, tag="pb", name="wps")
        for _ in range(24):
            nc.tensor.matmul(wps[:], lhsT=ident[:], rhs=ident[:],
                             start=True, stop=True)
        nc.vector.tensor_copy(obuf[:, 0, :], wps[:])

        # prologue: fast path to scatter(0); bases production (conv2h + do_C)
        # then runs TWO PAIRS AHEAD of the scatters so GPSIMD never starves.
        conv1_rows(0, 5)
        conv2h(0)
        do_C2(0)
        scatter_pair(0)
        conv1_rows(5, 3)
        conv1(1)
        conv2h(1)
        do_C2(1)
        scatter_pair(1)
        conv2h(2)
        do_C2(2)
        conv1(2)
        conv2h(3)
        do_C2(3)
        d_chunk(0, 2, pool=psO)
        d_chunk(2, 4)
        expand_pair(0)

        conv1_done = [2]

        def need_conv1(p):
            k = min(4 * p + 4, 63) // 8
            while conv1_done[0] < k:
                conv1_done[0] += 1
                conv1(conv1_done[0])

        for rh in range(2, NPAIR):
            scatter_pair(rh)
            # issue DMA transposes for the previous pair early (latency
            # hiding); late pairs lean more on PE (ring backlog at the end)
            expand_pair(rh - 1, pe_chunks=6 if rh >= 13 else PE_CHUNKS)
            # produce pair rh+2 (two ahead of the scatter queue: absorbs the
            # conv1-bunched iterations without starving GPSIMD)
            if rh + 2 < NPAIR:
                need_conv1(min(rh + 3, NPAIR - 1))
                conv2h(rh + 2)
                do_C2(rh + 2)
            # emits lag expansion by one pair
            emit_banded(2 * rh - 4)
            emit_banded(2 * rh - 3)
            d_chunk(2 * rh, 2 * rh + 2)
        # tail: pair-14 emits first (they only need pair-14 a3, already
        # expanded), overlapping the wait for scatter(15)
        emit_banded(NBLK - 4)
        emit_banded(NBLK - 3)
        expand_pair(NPAIR - 1, pe_chunks=12)  # last pair all-PE: shortest latency
        emit_banded(NBLK - 2)
        emit_banded(NBLK - 1)

    nc.compile()
    return nc


def _get_program():
    if "nc" not in _cached:
        _cached["nc"] = _build_program()
    return _cached["nc"]


def _build_in_maps(inputs, prep=None):
    if prep is None:
        prep = _host_prep(inputs)
    x = np.asarray(inputs["x"], _f32)

    shared = {
        "w1t": np.ascontiguousarray(prep["w1T"].reshape(C, 9 * INTER)),
        "s1": prep["s1"], "t1": prep["t1"],
        "w2t": np.ascontiguousarray(prep["w2T"].reshape(INTER, 9 * BS)),
        "s2": prep["s2"], "t2": prep["t2"],
        "fbbd": prep["fbbd"],
        "coeft": np.ascontiguousarray(prep["coefT"].reshape(C, M * O)),
        "idx4": prep["idx4"],
        "dummyidx": prep["dummyidx"],
        "ident": prep["ident"],
    }

    in_maps = []
    for n in range(N):
        m = dict(shared)
        m["xbf"] = np.ascontiguousarray(x[n].reshape(C, PIX).astype(_bf16))
        in_maps.append(m)
    return in_maps


def kernel(**inputs):
    from concourse.bass_utils import run_bass_kernel_spmd

    prep = _host_prep(inputs)
    in_maps = _build_in_maps(inputs, prep)

    nc = _get_program()
    res = run_bass_kernel_spmd(nc, in_maps, core_ids=list(range(N)))

    out = np.zeros((N, O, H, W), _f32)
    bias = prep["bias"]
    for n in range(N):
        outT = res.results[n]["out"]            # [4096, 128]
        out[n] = (outT.T + bias[:, None]).reshape(O, H, W)
    return out
